# revision 38
# baseline (speedup 1.0000x reference)
"""Trainium2 Bass kernel for CausalWanSelfAttention (sparse_attention).

Sharding: tensor-parallel over the 12-head axis across 8 cores, arranged as
4 pairs. Pair p covers heads {3p, 3p+1, 3p+2}: core 2p owns heads (3p, 3p+1),
core 2p+1 owns (3p+1, 3p+2) -- the middle head is computed redundantly by both
cores of the pair (M-tile padding makes this free on the PE), and its query
range is split 780/780 between them, so attention work is perfectly balanced:
every core runs 2340 query-rows x 10920 kv-rows of attention.

Key layout choices (all feature-major / "transposed", which makes every matmul
contraction land on the partition axis with zero on-chip transposes):
  - q^T/k^T/v computed from a host-shipped x^T; rope pairs are made
    partition-contiguous by a host-side even/odd feature permutation of the
    wq/wk columns (and, consistently, of the cached K rows).
  - S^T = K @ Q^T tiles [kv=128, q<=780] -> exp (no max subtraction needed:
    rmsnorm'd q,k bound |logit| <~ 12) -> P^T -> O^T = V^T-free accumulation.
  - softmax denominator: DVE elementwise tree over P^T tiles + one ones-matmul.
  - RMS-norm is over the full 1536 features (cross-head), so partial
    sum-of-squares takes one tiny [3120] AllReduce.
  - output projection: 3 staggered AllGathers of normalized O^T blocks, then
    each core computes its own 192 output columns; host concatenates.
All matmuls run in bf16 (fp32 PSUM accumulation). The two places where a
bf16 rounding would be a systematic (non-averaging) error -- the softmax
denominator reduce and the inv-rms / 1/denom broadcasts -- use a hi+lo bf16
split (two matmuls) so they are fp32-exact to ~1e-6.
"""

import numpy as np
import ml_dtypes
import concourse.bass as bass
import concourse.mybir as mybir
from concourse import tile
from concourse.bass_utils import run_bass_kernel_spmd

FP = mybir.dt.float32
BF = mybir.dt.bfloat16
I32 = mybir.dt.int32
AF = mybir.ActivationFunctionType
ALU = mybir.AluOpType

DIM, NH, HD = 1536, 12, 128
S = 1560                      # new tokens
CACHED = 9360                 # tokens already in cache (current_start)
KV = CACHED + S               # attention window length (w0 == 0)
EPS = 1e-6
SOFTMAX_SCALE = 1.0 / np.sqrt(HD)
NCORES = 8
LINEARIZE = False

# kv tiling: cached part in 5 DMA chunks, tiles of 128 (+ ragged)
CACHE_CHUNKS = [(0, 2048), (2048, 2048), (4096, 2048), (6144, 2048), (8192, 1168)]
RG = [list(range(8))]


def _replace_range_clear(nc):
    """This walrus build cannot encode EVENT_SEMAPHORE_RANGE_CLEAR (the tail
    semaphore reset Tile emits). All semaphore totals are static, so replace
    it with one sem-dec EventSemaphore per semaphore in the range."""
    import re
    f = nc.m.functions[0]
    totals = {}
    for b in f.blocks:
        for ins in b.instructions:
            si = ins.sync_info
            if si is None:
                continue
            for u in si.on_update:
                if u.sync_type == "semaphore":
                    d = u.update_value if u.update_mode == "sem-inc" else -u.update_value
                    totals[u.id] = totals.get(u.id, 0) + d
    for b in f.blocks:
        out = []
        for ins in b.instructions:
            tname = type(ins).__name__
            if tname == "InstNoOp":
                si2 = ins.sync_info
                if si2 is not None and (si2.on_wait or si2.on_update):
                    # CTRL_NO has no sync-wait slots in this walrus build;
                    # EVENT_SEMAPHORE is the encoding for sync-only ops
                    es = mybir.InstEventSemaphore(name=ins.name + "-es",
                                                  engine=ins.engine)
                    es.sync_info = si2
                    out.append(es)
                    continue
            if tname == "InstISA" and "RANGE_CLEAR" in ins.concise():
                # dropped: this NEFF is loaded+executed once per kernel()
                # call, so semaphores always start from zero
                continue
            if tname == "InstSeqAssert":
                # runtime bounds-assert from values_load; this walrus build
                # cannot encode it, and the qoff operand is host-controlled
                continue
            out.append(ins)
        b.instructions = out


def _legalize_waits(nc, dummy_sem, max_waits=1):
    """Split multi-wait instructions into same-engine single-wait NoOps + the
    original. This walrus build's codegen rejects >1 sync wait on many
    instruction encodings (DMA direct2d, memset, ldweights, ...); a NoOp
    chain in front is semantically identical -- the engine stalls on each
    semaphore in turn before executing the real instruction."""
    f = nc.m.functions[0]
    for b in f.blocks:
        out = []
        n = 0
        for ins in b.instructions:
            si = ins.sync_info
            if si is not None and len(si.on_wait) > max_waits:
                waits = list(si.on_wait)
                for w in waits[:-max_waits]:
                    nop = mybir.InstEventSemaphore(name=f"{ins.name}-wsplit{n}",
                                                   engine=ins.engine)
                    n += 1
                    nop.sync_info = mybir.SyncInfo(on_wait=[w], on_update=[
                        mybir.SyncUpdate(sync_type="semaphore", id=dummy_sem.num,
                                         ant_name=dummy_sem.name,
                                         update_mode="sem-inc", update_value=1)])
                    out.append(nop)
                ins.sync_info = mybir.SyncInfo(on_wait=waits[-max_waits:],
                                               on_update=list(si.on_update))
            out.append(ins)
        if n:
            b.instructions = out


def build_nc(legalize=True):
    nc = bass.Bass()

    # ---- per-core parameters (host-sharded) ----
    xT = nc.declare_dram_parameter("xT", [12, 128, S], BF, isOutput=False)
    wqT = nc.declare_dram_parameter("wqT", [12, 128, 256], BF, isOutput=False)
    wkT = nc.declare_dram_parameter("wkT", [12, 128, 256], BF, isOutput=False)
    wvT = nc.declare_dram_parameter("wvT", [12, 128, 256], BF, isOutput=False)
    woT = nc.declare_dram_parameter("woT", [12, 128, 192], BF, isOutput=False)
    bqs = nc.declare_dram_parameter("bqs", [1, 256], BF, isOutput=False)
    bks = nc.declare_dram_parameter("bks", [1, 256], BF, isOutput=False)
    bvr = nc.declare_dram_parameter("bvr", [1, 256], BF, isOutput=False)
    bos = nc.declare_dram_parameter("bos", [1, 192], BF, isOutput=False)
    ropec = nc.declare_dram_parameter("ropec", [128, S], FP, isOutput=False)
    ropes = nc.declare_dram_parameter("ropes", [128, 2 * S], FP, isOutput=False)
    ktc = nc.declare_dram_parameter("ktc", [2, 128, CACHED], BF, isOutput=False)
    vtc = nc.declare_dram_parameter("vtc", [2, CACHED, 128], BF, isOutput=False)
    qoff = nc.declare_dram_parameter("qoff", [1, 1], I32, isOutput=False)
    out = nc.declare_dram_parameter("out", [192, S], FP, isOutput=True)

    # ---- internal DRAM for collectives ----
    ssq_in = nc.dram_tensor("ssq_in", [1, 3120], FP)
    ssq_out = nc.dram_tensor("ssq_out", [1, 3120], FP, addr_space="Shared")
    ag_in = [nc.dram_tensor(f"ag_in{i}", [128, 780], BF) for i in range(3)]
    ag_out = [
        nc.dram_tensor(f"ag_out{i}", [NCORES, 128, 780], BF, addr_space="Shared")
        for i in range(3)
    ]

    with tile.TileContext(nc, num_cores=NCORES) as tc, \
            nc.allow_low_precision(reason="bf16 matmul operands; fp32 accumulation where it matters"):
        with tc.tile_pool(name="const", bufs=1) as cpool:
            junk = cpool.tile([1, 4], FP, tag="junk")

            def absorb_read(ap):
                # a gpsimd junk-copy: advances the gpsimd clock past ap's
                # writers so a following gpsimd DMA needs no wait for them
                nc.gpsimd.tensor_copy(junk[0:1, 0:1], ap)

            def absorb_write(ap):
                # gpsimd corner-memset: advances the gpsimd clock past ap's
                # readers (the value is about to be overwritten anyway)
                nc.gpsimd.memset(ap, 0.0)

            ones_col = cpool.tile([128, 1], BF, tag="ones_col")
            nc.vector.memset(ones_col[:], 1.0)
            ones_row = cpool.tile([1, 512], BF, tag="ones_row")
            nc.vector.memset(ones_row[:], 1.0)
            qoff_sb = cpool.tile([1, 1], I32, tag="qoff")
            nc.gpsimd.dma_start(qoff_sb[:], qoff[:])

            bq_sb = cpool.tile([1, 256], BF, tag="bq")
            bk_sb = cpool.tile([1, 256], BF, tag="bk")
            bv_sb = cpool.tile([1, 256], BF, tag="bv")
            bo_sb = cpool.tile([1, 192], BF, tag="bo")
            nc.gpsimd.dma_start(bq_sb[:], bqs[:])
            nc.gpsimd.dma_start(bk_sb[:], bks[:])
            nc.gpsimd.dma_start(bv_sb[:], bvr[:])
            nc.gpsimd.dma_start(bo_sb[:], bos[:])

            # persistent activations
            qT_sb = cpool.tile([128, 2 * S], BF, tag="qT")
            kT_sb = cpool.tile([128, 2 * S], BF, tag="kT")
            v_sb = cpool.tile([128, 13 * 256], BF, tag="v")
            rqB_sb = cpool.tile([128, 780], BF, tag="rqB")

            # whole per-head KV cache, resident in SBUF (bf16), loaded once
            kc_sb = []
            vc_sb = []
            for h in range(2):
                kch = cpool.tile([128, CACHED], BF, tag=f"kc{h}", name=f"kc{h}")
                vch = cpool.tile([128, 74, 128], BF, tag=f"vc{h}", name=f"vc{h}")
                kc_sb.append(kch)
                vc_sb.append(vch)
            for h in range(2):
                nc.gpsimd.dma_start(kc_sb[h][:], ktc[h])
                nc.gpsimd.dma_start(
                    vc_sb[h][:, 0:73, :],
                    vtc[h, 0:73 * 128, :].rearrange("(t p) d -> p t d", p=128),
                )
                nc.gpsimd.dma_start(vc_sb[h][0:16, 73, :], vtc[h, 73 * 128:CACHED, :])

            # ---------------- phase 1a: projections + ssq + v ----------------
            with (
                tc.tile_pool(name="p1a", bufs=1) as p1,
                tc.tile_pool(name="p1scr", bufs=2) as scr,
                tc.tile_pool(name="psqk", bufs=3, space="PSUM") as psqk,
                tc.tile_pool(name="psv", bufs=2, space="PSUM") as psv,
                tc.tile_pool(name="pssq", bufs=2, space="PSUM") as pssq,
            ):
                xT_sb = p1.tile([128, 12, S], BF, tag="xT")
                nc.gpsimd.dma_start(xT_sb[:], xT.rearrange("k p n -> p k n"))
                wq_sb = p1.tile([128, 12, 256], BF, tag="wq")
                wk_sb = p1.tile([128, 12, 256], BF, tag="wk")
                wv_sb = p1.tile([128, 12, 256], BF, tag="wv")
                nc.gpsimd.dma_start(wq_sb[:], wqT.rearrange("k p n -> p k n"))
                nc.gpsimd.dma_start(wk_sb[:], wkT.rearrange("k p n -> p k n"))
                nc.gpsimd.dma_start(wv_sb[:], wvT.rearrange("k p n -> p k n"))
                ssqrow_sb = p1.tile([1, 3120], FP, tag="ssqrow")

                # q^T, k^T projections: [256 feat, 1560 tok]
                for w_sb, t_sb, b_sb in ((wq_sb, qT_sb, bq_sb), (wk_sb, kT_sb, bk_sb)):
                    for m in range(2):
                        for nch in range(4):
                            ps = psqk.tile([128, 390], FP, tag="psqk")
                            for k in range(12):
                                nc.tensor.matmul(
                                    ps[:],
                                    w_sb[:, k, m * 128:(m + 1) * 128],
                                    xT_sb[:, k, nch * 390:(nch + 1) * 390],
                                    start=(k == 0), stop=False,
                                )
                            nc.tensor.matmul(
                                ps[:], b_sb[0:1, m * 128:(m + 1) * 128],
                                ones_row[0:1, 0:390], start=False, stop=True,
                            )
                            nc.vector.tensor_copy(
                                t_sb[:, m * S + nch * 390: m * S + (nch + 1) * 390],
                                ps[:],
                            )

                # partial sum-of-squares over this core's exclusive 192 features
                for row, t_sb in ((0, qT_sb), (1, kT_sb)):
                    for nch in range(4):
                        sl = slice(nch * 390, (nch + 1) * 390)
                        sq1 = scr.tile([128, 390], BF, tag="sq1")
                        nc.vector.tensor_mul(sq1[:], t_sb[:, sl], t_sb[:, sl])
                        sq2 = scr.tile([64, 390], BF, tag="sq2")
                        nc.vector.tensor_mul(
                            sq2[:], t_sb[0:64, S + nch * 390: S + (nch + 1) * 390],
                            t_sb[0:64, S + nch * 390: S + (nch + 1) * 390],
                        )
                        ps = pssq.tile([1, 390], FP, tag="pssq")
                        nc.tensor.matmul(ps[:], ones_col[:], sq1[:],
                                         start=True, stop=False)
                        nc.tensor.matmul(ps[:], ones_col[0:64, :], sq2[:],
                                         start=False, stop=True)
                        nc.vector.tensor_copy(
                            ssqrow_sb[0:1, row * S + nch * 390: row * S + (nch + 1) * 390],
                            ps[:],
                        )

                absorb_read(ssqrow_sb[0:1, 0:1])
                nc.gpsimd.dma_start(ssq_in[:], ssqrow_sb[:])
                nc.gpsimd.collective_compute(
                    "AllReduce", ALU.add, ins=[ssq_in[:]], outs=[ssq_out[:]],
                    replica_groups=RG,
                )

                # v projection (token-major), overlaps the AllReduce
                for tt in range(13):
                    nrow = 128 if tt < 12 else 24
                    ps = psv.tile([128, 256], FP, tag="psv")
                    for k in range(12):
                        nc.tensor.matmul(
                            ps[0:nrow, :],
                            xT_sb[:, k, tt * 128: tt * 128 + nrow],
                            wv_sb[:, k, :],
                            start=(k == 0), stop=False,
                        )
                    nc.tensor.matmul(
                        ps[0:nrow, :], ones_row[0:1, 0:nrow], bv_sb[:],
                        start=False, stop=True,
                    )
                    nc.vector.tensor_copy(
                        v_sb[0:nrow, tt * 256:(tt + 1) * 256], ps[0:nrow, :]
                    )

                # absorb this phase's engine ticks onto gpsimd before the
                # pool closes, so phase-1b DMAs landing in this region carry
                # no released-zone waits
                absorb_write(xT_sb[0:1, 0:1, 0:1])   # PE readers
                absorb_read(ssqrow_sb[0:1, 1:2])     # DVE writers

            # ---------------- phase 1b: rope + rms-norm scale ----------------
            with (
                tc.tile_pool(name="p1b", bufs=1) as p1b,
                tc.tile_pool(name="p1brt", bufs=2) as rscr,
                tc.tile_pool(name="pmisc1", bufs=1, space="PSUM") as pmisc1,
            ):
                rc_sb = p1b.tile([128, S], FP, tag="ropec")
                rs_sb = p1b.tile([128, 2 * S], FP, tag="ropes")
                nc.gpsimd.dma_start(rc_sb[:], ropec[:])
                nc.gpsimd.dma_start(rs_sb[:], ropes[:])
                invb_sb = p1b.tile([128, 2 * S], FP, tag="invb")

                # rope on q^T/k^T (in place):  rq = q*C + swap(q)*Ssigned
                # (swap = e/o 64-row block exchange via sbuf->sbuf DMA;
                # rotation signs folded into the host-built Ssigned table)
                qsw_t = []
                for i in range(4):
                    qswi = rscr.tile([128, S], BF, tag=f"qsw{i}", name=f"qsw{i}")
                    qsw_t.append(qswi)
                for seg in (0, 1):
                    csl = slice(seg * S, (seg + 1) * S)
                    for ti, t_sb in enumerate((qT_sb, kT_sb)):
                        qsw = qsw_t[seg * 2 + ti]
                        absorb_read(t_sb[0:1, csl][0:1, 0:1])
                        nc.gpsimd.dma_start(qsw[0:64, :], t_sb[64:128, csl])
                        nc.gpsimd.dma_start(qsw[64:128, :], t_sb[0:64, csl])
                        t5 = rscr.tile([128, S], BF, tag="t5")
                        nc.vector.tensor_mul(t5[:], t_sb[:, csl], rc_sb[:])
                        t6 = rscr.tile([128, S], BF, tag="t6")
                        nc.vector.tensor_mul(t6[:], qsw[:], rs_sb[:, csl])
                        nc.vector.tensor_add(t_sb[:, csl], t5[:], t6[:])

                # inv_rms = 1/sqrt(ssq/1536 + eps), hi/lo-bf16 broadcast
                ssqr_sb = p1b.tile([1, 3120], FP, tag="ssqr")
                nc.gpsimd.dma_start(ssqr_sb[:], ssq_out[:])
                eps_sb = p1b.tile([1, 1], FP, tag="eps")
                nc.vector.memset(eps_sb[:], EPS)
                sroot = p1b.tile([1, 3120], FP, tag="sroot")
                nc.scalar.activation(sroot[:], ssqr_sb[:], AF.Sqrt,
                                     bias=eps_sb[:], scale=1.0 / DIM)
                inv_f = ssqr_sb  # reuse the AR-readback tile (ping-pong)
                nc.vector.reciprocal(inv_f[:], sroot[:])
                i_hi = p1b.tile([1, 3120], BF, tag="ihi")
                nc.vector.tensor_copy(i_hi[:], inv_f[:])
                i_lo = p1b.tile([1, 3120], BF, tag="ilo")
                nc.vector.tensor_sub(i_lo[:], inv_f[:], i_hi[:])
                for ch in range(8):
                    csl8 = slice(ch * 390, (ch + 1) * 390)
                    ps = pmisc1.tile([128, 390], FP, tag="pmisc1")
                    nc.tensor.matmul(ps[:], ones_row[0:1, 0:128], i_hi[0:1, csl8],
                                     start=True, stop=False)
                    nc.tensor.matmul(ps[:], ones_row[0:1, 0:128], i_lo[0:1, csl8],
                                     start=False, stop=True)
                    nc.vector.tensor_copy(invb_sb[:, csl8], ps[:])

                # scale rq/rk by inv_rms (per token; both m-tiles)
                for t_sb, base in ((qT_sb, 0), (kT_sb, S)):
                    for m in range(2):
                        nc.vector.tensor_mul(
                            t_sb[:, m * S:(m + 1) * S],
                            t_sb[:, m * S:(m + 1) * S],
                            invb_sb[:, base:base + S],
                        )

                # gather this core's half of the shared head's queries
                sv = nc.values_load(qoff_sb[0:1, 0:1].to_broadcast((1, 1)))
                nc.vector.tensor_copy(rqB_sb[:], qT_sb[:, bass.ds(sv, 780)])

                # phase-boundary absorbers (PE read rope-region? no: DVE/ACT)
                absorb_read(invb_sb[0:1, 0:1])       # DVE writers
                absorb_read(sroot[0:1, 0:1])         # ACT writer

            # ---------------- phase 2: attention ----------------
            passes = [
                (qT_sb[:, 0:780], 0, 0),
                (qT_sb[:, 780:1560], 0, 1),
                (rqB_sb[:, 0:780], 1, 2),
            ]
            with (
                tc.tile_pool(name="p2sb", bufs=2) as p2,
                tc.tile_pool(name="p2p", bufs=3) as pp,
                tc.tile_pool(name="pS", bufs=2, space="PSUM") as pS,
                tc.tile_pool(name="pO", bufs=1, space="PSUM") as pO,
                tc.tile_pool(name="pmisc2", bufs=1, space="PSUM") as pmisc2,
            ):
                last_P = None
                last_onorm = None
                for qsrc, h, agi in passes:
                    O_ps = pO.tile([128, 1024], FP, tag="O")
                    dacc = p2.tile([128, 780], FP, tag="dacc")

                    tiles_meta = []
                    for ti in range(74):
                        kv_t = 128 if ti < 73 else 16
                        tiles_meta.append(
                            (kv_t, kc_sb[h][:, ti * 128: ti * 128 + kv_t],
                             vc_sb[h][0:kv_t, ti, :])
                        )
                    for ti in range(13):
                        kv_t = 128 if ti < 12 else 24
                        tiles_meta.append(
                            (kv_t,
                             kT_sb[:, h * S + ti * 128: h * S + ti * 128 + kv_t],
                             v_sb[0:kv_t, ti * 256 + h * 128: ti * 256 + (h + 1) * 128])
                        )

                    nt = len(tiles_meta)
                    for t, (kv_t, k_ap, v_ap) in enumerate(tiles_meta):
                        S_ps = pS.tile([128, 1024], FP, tag="S")
                        nc.tensor.matmul(S_ps[0:kv_t, 0:512], k_ap,
                                         qsrc[:, 0:512], start=True, stop=True)
                        nc.tensor.matmul(S_ps[0:kv_t, 512:780], k_ap,
                                         qsrc[:, 512:780], start=True, stop=True)
                        P_t = pp.tile([128, 780], BF, tag="P")
                        nc.scalar.activation(P_t[0:kv_t, :], S_ps[0:kv_t, 0:780],
                                             AF.Exp, scale=SOFTMAX_SCALE)
                        nc.tensor.matmul(O_ps[:, 0:512], v_ap,
                                         P_t[0:kv_t, 0:512],
                                         start=(t == 0), stop=(t == nt - 1))
                        nc.tensor.matmul(O_ps[:, 512:780], v_ap,
                                         P_t[0:kv_t, 512:780],
                                         start=(t == 0), stop=(t == nt - 1))
                        if t == 0:
                            nc.vector.tensor_copy(dacc[:], P_t[:])
                        else:
                            nc.vector.tensor_add(dacc[0:kv_t, :], dacc[0:kv_t, :],
                                                 P_t[0:kv_t, :])
                        last_P = P_t

                    # hi+lo bf16 split keeps the fp32 dacc reduce exact
                    d_hi = p2.tile([128, 780], BF, tag="dhi")
                    nc.vector.tensor_copy(d_hi[:], dacc[:])
                    d_lo = p2.tile([128, 780], BF, tag="dlo")
                    nc.vector.tensor_sub(d_lo[:], dacc[:], d_hi[:])
                    d_ps = pmisc2.tile([1, 1024], FP, tag="dB")
                    for lo_, hi_ in ((0, 512), (512, 780)):
                        nc.tensor.matmul(d_ps[0:1, lo_:hi_], ones_col[:],
                                         d_hi[:, lo_:hi_], start=True, stop=False)
                        nc.tensor.matmul(d_ps[0:1, lo_:hi_], ones_col[:],
                                         d_lo[:, lo_:hi_], start=False, stop=True)
                    recip_f = p2.tile([1, 780], FP, tag="recipf")
                    nc.vector.reciprocal(recip_f[:], d_ps[0:1, 0:780])
                    r_hi = p2.tile([1, 780], BF, tag="rhi")
                    nc.vector.tensor_copy(r_hi[:], recip_f[:])
                    r_lo = p2.tile([1, 780], BF, tag="rlo")
                    nc.vector.tensor_sub(r_lo[:], recip_f[:], r_hi[:])
                    B_ps = pmisc2.tile([128, 1024], FP, tag="dB")
                    for lo_, hi_ in ((0, 512), (512, 780)):
                        nc.tensor.matmul(B_ps[:, lo_:hi_], ones_row[0:1, 0:128],
                                         r_hi[0:1, lo_:hi_], start=True, stop=False)
                        nc.tensor.matmul(B_ps[:, lo_:hi_], ones_row[0:1, 0:128],
                                         r_lo[0:1, lo_:hi_], start=False, stop=True)
                    bnorm = p2.tile([128, 780], FP, tag="bnorm")
                    nc.vector.tensor_copy(bnorm[:], B_ps[:, 0:780])
                    onorm = p2.tile([128, 780], BF, tag="onorm")
                    nc.vector.tensor_mul(onorm[:], O_ps[:, 0:780], bnorm[:])
                    last_onorm = onorm
                    absorb_read(onorm[0:1, 0:1])
                    nc.gpsimd.dma_start(ag_in[agi][:], onorm[:])
                    nc.gpsimd.collective_compute(
                        "AllGather", ALU.bypass, ins=[ag_in[agi][:]],
                        outs=[ag_out[agi][:]], replica_groups=RG,
                    )

                # phase-boundary absorbers for the freed attention pools
                absorb_read(last_P[0:1, 0:1])        # ACT writer of P
                absorb_write(last_P[0:1, 0:1])       # PE readers of P
                absorb_read(last_onorm[0:1, 0:1])    # DVE writers

            # ---------------- phase 3: output projection ----------------
            with (
                tc.tile_pool(name="p3sb", bufs=1) as p3,
                tc.tile_pool(name="pop", bufs=8, space="PSUM") as pop,
            ):
                wo_sb = p3.tile([128, 12, 192], BF, tag="wo")
                nc.gpsimd.dma_start(wo_sb[:], woT.rearrange("k p n -> p k n"))
                of_sb = p3.tile([128, 12, S], BF, tag="of")
                for hh in range(12):
                    p_, r_ = hh // 3, hh % 3
                    if r_ == 0:
                        for half in range(2):
                            nc.gpsimd.dma_start(of_sb[:, hh, half * 780:(half + 1) * 780],
                                                ag_out[half][2 * p_])
                    elif r_ == 2:
                        for half in range(2):
                            nc.gpsimd.dma_start(of_sb[:, hh, half * 780:(half + 1) * 780],
                                                ag_out[half][2 * p_ + 1])
                    else:
                        nc.gpsimd.dma_start(of_sb[:, hh, 0:780], ag_out[2][2 * p_])
                        nc.gpsimd.dma_start(of_sb[:, hh, 780:1560], ag_out[2][2 * p_ + 1])

                for m, mrow in ((0, 128), (1, 64)):
                    for nch in range(4):
                        ps = pop.tile([128, 390], FP, tag="pop")
                        for k in range(12):
                            nc.tensor.matmul(
                                ps[0:mrow, :],
                                wo_sb[:, k, m * 128: m * 128 + mrow],
                                of_sb[:, k, nch * 390:(nch + 1) * 390],
                                start=(k == 0), stop=False,
                            )
                        nc.tensor.matmul(
                            ps[0:mrow, :], bo_sb[0:1, m * 128: m * 128 + mrow],
                            ones_row[0:1, 0:390], start=False, stop=True,
                        )
                        ev = p3.tile([128, 390], FP, tag=f"ev{m}_{nch}")
                        nc.vector.tensor_copy(ev[0:mrow, :], ps[0:mrow, :])
                        absorb_read(ev[0:1, 0:1])
                        nc.gpsimd.dma_start(
                            out[m * 128: m * 128 + mrow,
                                nch * 390:(nch + 1) * 390],
                            ev[0:mrow, :],
                        )

    if legalize:
        dummy_sem = nc.alloc_semaphore("wsplit_dummy")
        _legalize_waits(nc, dummy_sem)
        _replace_range_clear(nc)
    return nc


# ============================ host-side sharding ============================

def _token_tables_np(freqs_cos, freqs_sin, grid, current_start):
    C = HD // 2
    cb = C - 2 * (C // 3)
    c3 = C // 3
    F, H, W = int(grid[0]), int(grid[1]), int(grid[2])
    start_frame = int(current_start) // (H * W)

    def grid_tab(tab):
        out = np.empty((F, H, W, C), np.float32)
        out[..., :cb] = tab[start_frame:start_frame + F, None, None, :cb]
        out[..., cb:cb + c3] = tab[None, :H, None, cb:cb + c3]
        out[..., cb + c3:] = tab[None, None, :W, cb + c3:cb + 2 * c3]
        return out.reshape(F * H * W, C)

    return grid_tab(np.asarray(freqs_cos)), grid_tab(np.asarray(freqs_sin))


def shard_inputs(inputs):
    x = np.asarray(inputs["x"], np.float32)
    wq, bq = np.asarray(inputs["wq"]), np.asarray(inputs["bq"])
    wk, bk = np.asarray(inputs["wk"]), np.asarray(inputs["bk"])
    wv, bv = np.asarray(inputs["wv"]), np.asarray(inputs["bv"])
    wo, bo = np.asarray(inputs["wo"]), np.asarray(inputs["bo"])
    gq, gk = np.asarray(inputs["gq"]), np.asarray(inputs["gk"])
    cache_k = np.asarray(inputs["cache_k"])
    cache_v = np.asarray(inputs["cache_v"])
    grid = np.asarray(inputs["grid_sizes"]).reshape(-1)[:3]
    current_start = int(np.asarray(inputs["current_start"]))
    assert current_start == CACHED, "kernel compiled for current_start == 9360"

    wq_eff = (gq[:, None] * wq).astype(np.float32)
    wk_eff = (gk[:, None] * wk).astype(np.float32)
    bq_eff = (gq * bq).astype(np.float32)
    bk_eff = (gk * bk).astype(np.float32)

    tcos, tsin = _token_tables_np(inputs["freqs_cos"], inputs["freqs_sin"],
                                  grid, current_start)
    cT = np.ascontiguousarray(tcos.T)   # [64, 1560]
    sT = np.ascontiguousarray(tsin.T)

    perm_even = np.concatenate([np.arange(0, 128, 2), np.arange(1, 128, 2)])
    perm_odd = np.concatenate([np.arange(1, 128, 2), np.arange(0, 128, 2)])

    xT_full = np.ascontiguousarray(x[0].T).reshape(12, 128, S)

    in_maps = []
    for c in range(NCORES):
        p, parity = c // 2, c % 2
        hA = 3 * p if parity == 0 else 3 * p + 2
        hB = 3 * p + 1
        permB = perm_even if parity == 0 else perm_odd
        featA = hA * 128 + perm_even
        featB = hB * 128 + permB
        featqk = np.concatenate([featA, featB])
        featv = np.concatenate([np.arange(hA * 128, hA * 128 + 128),
                                np.arange(hB * 128, hB * 128 + 128)])
        jcols = np.arange(192 * c, 192 * c + 192)

        ssA = np.concatenate([-sT, sT], axis=0)
        ssB = ssA if parity == 0 else -ssA


        bf = ml_dtypes.bfloat16
        m = {
            "xT": xT_full.astype(bf),
            "wqT": np.ascontiguousarray(wq_eff[featqk, :].T).reshape(12, 128, 256).astype(bf),
            "wkT": np.ascontiguousarray(wk_eff[featqk, :].T).reshape(12, 128, 256).astype(bf),
            "wvT": np.ascontiguousarray(wv[featv, :].T).reshape(12, 128, 256).astype(bf),
            "woT": np.ascontiguousarray(wo[jcols, :].T).reshape(12, 128, 192).astype(bf),
            "bqs": np.ascontiguousarray(bq_eff[featqk][None, :]).astype(bf),
            "bks": np.ascontiguousarray(bk_eff[featqk][None, :]).astype(bf),
            "bvr": np.ascontiguousarray(bv[featv][None, :]).astype(bf),
            "bos": np.ascontiguousarray(bo[jcols][None, :]).astype(bf),
            "ropec": np.ascontiguousarray(np.concatenate([cT, cT], axis=0)),
            "ropes": np.ascontiguousarray(np.concatenate([ssA, ssB], axis=1)),
            "ktc": np.ascontiguousarray(np.stack([
                cache_k[0, :CACHED, hA, :].T[perm_even],
                cache_k[0, :CACHED, hB, :].T[permB],
            ])).astype(bf),
            "vtc": np.ascontiguousarray(np.stack([
                cache_v[0, :CACHED, hA, :],
                cache_v[0, :CACHED, hB, :],
            ])).astype(bf),
            "qoff": np.array([[S + 780 * parity]], np.int32),
        }
        in_maps.append({k: np.ascontiguousarray(v, dtype=v.dtype) for k, v in m.items()})
    return in_maps


def unshard(results):
    full = np.zeros((1, S, DIM), np.float32)
    for c in range(NCORES):
        full[0, :, 192 * c:192 * (c + 1)] = results[c]["out"].T
    return full


def kernel(**inputs):
    nc = build_nc()
    in_maps = shard_inputs(inputs)
    res = run_bass_kernel_spmd(nc, in_maps, core_ids=list(range(NCORES)))
    return unshard(res.results)


if __name__ == "__main__":
    print("building nc ...")
    nc = build_nc()
    print("ok:", len(nc.m.functions[0].instructions)
          if hasattr(nc.m.functions[0], "instructions") else "built")
